# revision 31
# baseline (speedup 1.0000x reference)
"""Trainium2 Bass kernel for nn_AttentionLayer (linear attention, sparse_attention).

Math (per batch element n of B*H*W=2304):
    q = qin @ Wq + bq                (N=80 tokens, C=256 -> 128)
    k = [x|guidance] @ Wk + bk       (S=77 tokens)
    v = x @ Wv + bv
    Q = elu(q)+1, K = elu(k)+1       (8 heads x 16 dim)
    ZD[l,h]   = sum_d Q[l,hd]*Ksum[hd],  Ksum = sum_s K
    Z         = 1/(ZD+eps)
    Qbar[hd]  = sum_l Q[l,hd]*Z[l,h]
    A[h,s]    = sum_d Qbar[hd]*K[s,hd]
    out[hv]   = (1/N) * sum_s A[h,s]*v[s,hv]     (exact refactor of reference)

Feature layout: feature dim (128) on SBUF partitions, tokens on the free axis.
Host supplies pre-transposed inputs; output is transposed back on host.

v16: fp8 q AND k projections (DoubleRow, one matmul each); v projection from
bf16 x (fp8 v was outside tolerance). Supergroup-batched epilogue: qfm/kfm/
vsb and the two product tensors are supergroup-wide tiles so each segmented
reduce / mask-broadcast issues once per 24 batch elements instead of four
times, and the k-side tensors are padded to an 80-token pitch so a pairwise
halving add (bf16 2x mode) runs before every 1x tensor_reduce.

Engine split:
  ACT    : exp+relu of q/k (bias fused), v copy (+bias), A^T copy
  GPSIMD : fm adds (min(exp,1)+relu), kbd/abd mask*broadcast products
  DVE    : min(exp,1), halving adds + segmented reduces, fast recip,
           products against expander outputs
  PE     : projections (fp8 DR), ZD/A small matmuls, Z/A expanders
"""

import numpy as np
import ml_dtypes

import concourse.bass as bass
import concourse.bacc as bacc
import concourse.mybir as mybir
import concourse.tile as tile
from concourse.bass_utils import run_bass_kernel_spmd
from concourse.dve_ops import RECIP_APPROX_FAST_CONSTS, RECIPROCAL_APPROX_FAST

F32 = mybir.dt.float32
BF16 = mybir.dt.bfloat16
FP8 = mybir.dt.float8e4
AF = mybir.ActivationFunctionType
ALU = mybir.AluOpType
DR = mybir.MatmulPerfMode.DoubleRow


NCORES = 8
NH, DH, HID = 8, 16, 128
S, NTOK, C = 77, 80, 256
SP = 80                          # padded k-side pitch (aligned halving)
B, N_, H_, W_ = 4, 80, 24, 24
NTOTAL = B * H_ * W_            # 2304
NLOC = NTOTAL // NCORES         # 288
GRP = 6                         # batch elements per group
NGRP_SUPER = 4                  # groups per supergroup (4 x 8 rows in PSUM bands)
SUPER = GRP * NGRP_SUPER        # 24 n per supergroup


def build_nc(n_loc: int = NLOC) -> bass.Bass:
    assert n_loc % SUPER == 0
    nsuper = n_loc // SUPER

    nc = bacc.Bacc()

    qt = nc.declare_dram_parameter("qt", [2, HID, n_loc * NTOK], FP8, isOutput=False)
    xg = nc.declare_dram_parameter("xg", [2, HID, n_loc * S], FP8, isOutput=False)
    xt = nc.declare_dram_parameter("xt", [HID, n_loc * S], BF16, isOutput=False)
    c8p = nc.declare_dram_parameter("c8", [HID, 4 * HID], FP8, isOutput=False)
    CBW = 2 * HID + 2 * SUPER * NH + 16   # wv|e8|m1|mn|f32bits
    cbp = nc.declare_dram_parameter("cb", [HID, CBW], BF16, isOutput=False)
    out = nc.declare_dram_parameter("o", [HID, n_loc], F32, isOutput=True)

    QF = GRP * NTOK   # 480 free elems per group (q side)
    KF = GRP * S      # 462 free elems per group (k/v side)
    QS = SUPER * NTOK  # 1920 per supergroup
    KS = SUPER * S     # 1848

    rc = RECIP_APPROX_FAST_CONSTS

    with tile.TileContext(nc) as tc:
        with (
            tc.tile_pool(name="consts", bufs=1) as consts,
            tc.tile_pool(name="dmain", bufs=3) as dmain,
            tc.tile_pool(name="work", bufs=6) as work,
            tc.tile_pool(name="persist", bufs=3) as persist,
            tc.tile_pool(name="prod", bufs=2) as prod,
            tc.tile_pool(name="za", bufs=3) as za,
            tc.tile_pool(name="small", bufs=8) as small,
            tc.tile_pool(name="outp", bufs=1) as outp,
            tc.tile_pool(name="pq", bufs=1, space="PSUM") as pqp,
            tc.tile_pool(name="pk", bufs=1, space="PSUM") as pkp,
            tc.tile_pool(name="pv", bufs=1, space="PSUM") as pvp,
            tc.tile_pool(name="pzd", bufs=1, space="PSUM") as pzdp,
            tc.tile_pool(name="pe", bufs=3, space="PSUM") as pep,
            tc.tile_pool(name="pa", bufs=1, space="PSUM") as pap,
        ):
            # ---- constants: two packed blobs, two DMAs ----
            cb_t = consts.tile([HID, CBW], BF16)
            nc.sync.dma_start(cb_t[:], cbp[:])
            c8_t = consts.tile([HID, 4 * HID], FP8)
            nc.sync.dma_start(c8_t[:], c8p[:])
            # DoubleRow stationaries: [Ki=128, Ko=2, M=128]
            wq_dr = c8_t[:, 0:2 * HID].rearrange("p (ko m) -> p ko m", m=HID)
            wk_dr = c8_t[:, 2 * HID:4 * HID].rearrange("p (ko m) -> p ko m", m=HID)
            wv_t = cb_t[:, 0:HID]
            e8_t = cb_t[:, HID:2 * HID]
            m1_t = cb_t[:, 2 * HID:2 * HID + SUPER * NH]
            mn_t = cb_t[:, 2 * HID + SUPER * NH:2 * HID + 2 * SUPER * NH]
            fb_t = cb_t[:, CBW - 16:CBW].bitcast(F32)
            bq_t = fb_t[:, 0:1]
            bk_t = fb_t[:, 1:2]
            bv_t = fb_t[:, 2:3]
            bq1_t = fb_t[:, 3:4]   # bq + 1
            bk1_t = fb_t[:, 4:5]   # bk + 1
            tc.strict_bb_all_engine_barrier()

            # HAM warm-up: chained garbage matmuls keep the PE busy >3.4us so
            # the clock gate opens before real work; overlaps the first input
            # DMA. (Steady state re-throttles regardless; this helps the head.)
            heat = pep.tile([HID, 512], F32, tag="pze")
            for h in range(14):
                nc.tensor.matmul(heat[:], c8_t[:, 0:HID], c8_t[:, 0:512],
                                 start=(h == 0), stop=(h == 13))

            outT = outp.tile([HID, n_loc], F32)

            for sg in range(nsuper):
                # ---- supergroup DMA in (fp8) ----
                qt_sb = dmain.tile([HID, 2, QS], FP8, tag="qt")
                kg_sb = dmain.tile([HID, 2, KS], FP8, tag="kg")
                xt_sb = dmain.tile([HID, KS], BF16, tag="xt")
                nc.sync.dma_start(xt_sb[:], xt[:, sg * KS:(sg + 1) * KS])
                for po in range(2):
                    nc.sync.dma_start(
                        qt_sb[:, po, :], qt[po, :, sg * QS:(sg + 1) * QS]
                    )
                    nc.sync.dma_start(
                        kg_sb[:, po, :], xg[po, :, sg * KS:(sg + 1) * KS]
                    )

                # supergroup-lifetime PSUM bands (4 groups x 8 rows each)
                pzd = pzdp.tile([HID, 512], F32, tag="pzd")
                pzd = pzd[:, :QF]
                pa = pap.tile([HID, 512], F32, tag="pa")
                pa = pa[:, :KF]

                # supergroup-wide epilogue tiles; kfm uses an 80-token pitch
                # (cols S..SP-1 zeroed) so halving adds stay 4B-aligned.
                qfm = persist.tile([HID, SUPER, NTOK], BF16, tag="qfm")
                kfm = persist.tile([HID, SUPER, SP], BF16, tag="kfm")
                vsb = persist.tile([HID, KS], BF16, tag="vsb")

                # ================= front half: proj + fm =================
                for g in range(NGRP_SUPER):
                    qs = slice(g * QF, (g + 1) * QF)
                    ks = slice(g * KF, (g + 1) * KF)
                    gn = slice(g * GRP, (g + 1) * GRP)

                    pq = pqp.tile([HID, 512], F32, tag="pq")
                    pq = pq[:, :QF]
                    pk = pkp.tile([HID, 512], F32, tag="pk")
                    pk = pk[:, :KF]
                    pv = pvp.tile([HID, 512], F32, tag="pv")
                    pv = pv[:, :KF]
                    nc.tensor.matmul(pq[:], wq_dr, qt_sb[:, :, qs],
                                     start=True, stop=True, perf_mode=DR)
                    nc.tensor.matmul(pk[:], wk_dr, kg_sb[:, :, ks],
                                     start=True, stop=True, perf_mode=DR)
                    nc.tensor.matmul(pv[:], wv_t, xt_sb[:, ks],
                                     start=True, stop=True)

                    # feature map: fm(y) = min(exp(y), 1) + relu(y)
                    eq = work.tile([HID, QF], BF16, tag="eq")
                    rq = work.tile([HID, QF], BF16, tag="rq")
                    ek = work.tile([HID, KF], BF16, tag="ek")
                    rk = work.tile([HID, KF], BF16, tag="rk")
                    nc.scalar.activation(eq[:], pq[:], AF.Exp, bias=bq_t)
                    nc.scalar.activation(rq[:], pq[:], AF.Relu, bias=bq_t)
                    nc.scalar.activation(ek[:], pk[:], AF.Exp, bias=bk_t)
                    nc.scalar.activation(rk[:], pk[:], AF.Relu, bias=bk_t)
                    nc.scalar.activation(
                        vsb[:, ks].rearrange("p (g s) -> p g s", s=S),
                        pv[:].rearrange("p (g s) -> p g s", s=S),
                        AF.Identity, bias=bv_t)
                    eqm = work.tile([HID, QF], BF16, tag="eqm")
                    ekm = work.tile([HID, KF], BF16, tag="ekm")
                    nc.vector.tensor_scalar_min(eqm[:], eq[:], 1.0)
                    nc.vector.tensor_scalar_min(ekm[:], ek[:], 1.0)
                    nc.gpsimd.tensor_tensor(
                        qfm[:, gn, :].rearrange("p g l -> p (g l)"),
                        eqm[:], rq[:], ALU.add)
                    nc.gpsimd.tensor_tensor(
                        kfm[:, gn, 0:S],
                        ekm[:].rearrange("p (g s) -> p g s", s=S),
                        rk[:].rearrange("p (g s) -> p g s", s=S), ALU.add)

                # ===== supergroup: Ksum, one batched reduce =====
                ksum = small.tile([HID, SUPER], F32, tag="ksum")
                nc.vector.tensor_reduce(
                    ksum[:], kfm[:, :, 0:S], mybir.AxisListType.X, ALU.add)
                # KBD = mask1 * Ksum  (8 cols per n, whole supergroup)
                kbd = small.tile([HID, SUPER * NH], BF16, tag="kbd")
                nc.gpsimd.tensor_tensor(
                    kbd[:].rearrange("p (g h) -> p g h", h=NH),
                    m1_t[:].rearrange("p (g h) -> p g h", h=NH),
                    ksum[:, :, None].to_broadcast((HID, SUPER, NH)),
                    ALU.mult)

                # ZD rows -> 8-row bands at partition base 32*g
                for gi in range(SUPER):
                    g = gi // GRP
                    i = gi % GRP
                    nc.tensor.matmul(
                        pzd[32 * g:32 * g + NH, i * NTOK:(i + 1) * NTOK],
                        kbd[:, gi * NH:(gi + 1) * NH],
                        qfm[:, gi, :],
                        start=True, stop=True, skip_group_check=True,
                        tile_position=(0, 32 * g))

                # ========== supergroup: Z ~= 1/ZD (fast recip, bf16 out) =====
                zpk = za.tile([HID, QF], BF16, tag="zpk")
                nc.vector._custom_dve(
                    RECIPROCAL_APPROX_FAST, out=zpk[:], in0=pzd[:],
                    s0=rc["s0"], s1=rc["s1"], imm2=rc["imm2"])

                # ================= back half =================
                prodq = prod.tile([HID, SUPER, NTOK], BF16, tag="prodq")
                for g in range(NGRP_SUPER):
                    gn = slice(g * GRP, (g + 1) * GRP)
                    rowg = slice(32 * g, 32 * g + NH)
                    # Zexp (128, 480): one expander matmul per group
                    pze = pep.tile([HID, 512], F32, tag="pze")
                    pze = pze[:, :QF]
                    nc.tensor.matmul(
                        pze[:], e8_t[rowg, :], zpk[rowg, :],
                        start=True, stop=True, tile_position=(32 * g, 0))
                    nc.vector.tensor_tensor(
                        prodq[:, gn, :],
                        qfm[:, gn, :],
                        pze[:].rearrange("p (g l) -> p g l", l=NTOK),
                        ALU.mult)

                # Qbar: one batched reduce for the whole supergroup
                qbar = small.tile([HID, SUPER], F32, tag="qbar")
                nc.vector.tensor_reduce(
                    qbar[:], prodq[:], mybir.AxisListType.X, ALU.add)

                # Abd = maskn * Qbar (whole supergroup)
                abd = small.tile([HID, SUPER * NH], BF16, tag="abd")
                nc.gpsimd.tensor_tensor(
                    abd[:].rearrange("p (g h) -> p g h", h=NH),
                    mn_t[:].rearrange("p (g h) -> p g h", h=NH),
                    qbar[:, :, None].to_broadcast((HID, SUPER, NH)),
                    ALU.mult)

                # A^T rows -> 8-row bands at partition base 32*g
                for gi in range(SUPER):
                    g = gi // GRP
                    i = gi % GRP
                    nc.tensor.matmul(
                        pa[32 * g:32 * g + NH, i * S:(i + 1) * S],
                        abd[:, gi * NH:(gi + 1) * NH],
                        kfm[:, gi, 0:S],
                        start=True, stop=True, skip_group_check=True,
                        tile_position=(0, 32 * g))

                # A^T -> SBUF bf16 once per supergroup
                apk = za.tile([HID, KF], BF16, tag="apk")
                nc.scalar.activation(apk[:], pa[:], AF.Copy)

                prodv = prod.tile([HID, SUPER, SP], BF16, tag="prodv")
                for g in range(NGRP_SUPER):
                    gn = slice(g * GRP, (g + 1) * GRP)
                    ks = slice(g * KF, (g + 1) * KF)
                    rowg = slice(32 * g, 32 * g + NH)
                    pae = pep.tile([HID, 512], F32, tag="pze")
                    pae = pae[:, :KF]
                    nc.tensor.matmul(
                        pae[:], e8_t[rowg, :], apk[rowg, :],
                        start=True, stop=True, tile_position=(32 * g, 0))
                    nc.vector.tensor_tensor(
                        prodv[:, gn, 0:S],
                        vsb[:, ks].rearrange("p (g s) -> p g s", s=S),
                        pae[:].rearrange("p (g s) -> p g s", s=S),
                        ALU.mult)

                # out: one batched reduce for the whole supergroup
                ocol = sg * SUPER
                nc.vector.tensor_reduce(
                    outT[:, ocol:ocol + SUPER], prodv[:, :, 0:S],
                    mybir.AxisListType.X, ALU.add)

            nc.sync.dma_start(out[:], outT[:])

    nc.finalize()
    return nc


# ---------------- host-side packing ----------------

def make_consts():
    hd = np.arange(HID)
    e8 = (hd[None, :] // DH == (np.arange(HID) % NH)[:, None]).astype(np.float32)
    m1 = np.zeros((HID, SUPER * NH), np.float32)
    for i in range(SUPER):
        for h in range(NH):
            m1[h * DH:(h + 1) * DH, i * NH + h] = 1.0
    mn = (m1 / float(NTOK)).astype(np.float32)
    return e8, m1, mn


def shard_inputs(query, x, guidance, Wq, bq, Wk, bk, Wv, bv, n_loc=NLOC,
                 ncores=NCORES):
    qin = np.ascontiguousarray(
        query.transpose(0, 2, 3, 1, 4)).reshape(NTOTAL, NTOK, C)
    e8, m1, mn = make_consts()
    bf = ml_dtypes.bfloat16
    f8 = ml_dtypes.float8_e4m3
    wqr = Wq.reshape(2, HID, HID)
    wkr = Wk.reshape(2, HID, HID)
    # DoubleRow stationaries [Ki, Ko, M] flattened to [Ki, Ko*M]
    wq_dr = np.stack([wqr[0], wqr[1]], axis=1).reshape(HID, 2 * HID)
    wk_dr = np.stack([wkr[0], wkr[1]], axis=1).reshape(HID, 2 * HID)
    c8 = np.concatenate([wq_dr, wk_dr], axis=1).astype(f8)
    cb = np.concatenate([Wv, e8, m1, mn], axis=1).astype(bf)
    z = np.zeros(HID, np.float32)
    fb = np.stack(
        [bq, bk, bv, bq + 1.0, bk + 1.0, z, z, z], axis=1).astype(np.float32)
    fb_as_bf = np.ascontiguousarray(fb).view(bf)
    cb = np.concatenate([cb, fb_as_bf], axis=1)
    shared = dict(cb=cb, c8=c8)
    in_maps = []
    for i in range(ncores):
        sl = slice(i * n_loc, (i + 1) * n_loc)
        qc = qin[sl].reshape(n_loc * NTOK, C)
        xc = x[sl].reshape(n_loc * S, HID)
        gc = guidance[sl].reshape(n_loc * S, HID)
        m = dict(shared)
        m["qt"] = np.ascontiguousarray(qc.T).reshape(2, HID, n_loc * NTOK).astype(f8)
        xct = np.ascontiguousarray(xc.T)
        xgs = np.stack([xct, np.ascontiguousarray(gc.T)])
        m["xg"] = xgs.astype(f8)
        m["xt"] = xct.astype(bf)
        in_maps.append(m)
    return in_maps


_NC_CACHE = {}


def kernel(**inputs) -> np.ndarray:
    inputs = {k: np.asarray(v, dtype=np.float32) if np.asarray(v).dtype != np.int32
              else np.asarray(v) for k, v in inputs.items()}
    in_maps = shard_inputs(**inputs)
    if NLOC not in _NC_CACHE:
        _NC_CACHE[NLOC] = build_nc(NLOC)
    nc = _NC_CACHE[NLOC]
    res = run_bass_kernel_spmd(nc, in_maps, core_ids=list(range(NCORES)))
    outs = [np.asarray(res.results[i]["o"]).T for i in range(NCORES)]
    full = np.concatenate(outs, axis=0)  # (2304, 128)
    return full.reshape(B, H_, W_, HID).astype(np.float32)


# revision 32
# speedup vs baseline: 1.0280x; 1.0280x over previous
"""Trainium2 Bass kernel for nn_AttentionLayer (linear attention, sparse_attention).

Math (per batch element n of B*H*W=2304):
    q = qin @ Wq + bq                (N=80 tokens, C=256 -> 128)
    k = [x|guidance] @ Wk + bk       (S=77 tokens)
    v = x @ Wv + bv
    Q = elu(q)+1, K = elu(k)+1       (8 heads x 16 dim)
    ZD[l,h]   = sum_d Q[l,hd]*Ksum[hd],  Ksum = sum_s K
    Z         = 1/(ZD+eps)
    Qbar[hd]  = sum_l Q[l,hd]*Z[l,h]
    A[h,s]    = sum_d Qbar[hd]*K[s,hd]
    out[hv]   = (1/N) * sum_s A[h,s]*v[s,hv]     (exact refactor of reference)

Feature layout: feature dim (128) on SBUF partitions, tokens on the free axis.
Host supplies pre-transposed inputs; output is transposed back on host.

v16: fp8 q AND k projections (DoubleRow, one matmul each); v projection from
bf16 x (fp8 v was outside tolerance). Supergroup-batched epilogue: qfm/kfm/
vsb and the two product tensors are supergroup-wide tiles so each segmented
reduce / mask-broadcast issues once per 24 batch elements instead of four
times, and the k-side tensors are padded to an 80-token pitch so a pairwise
halving add (bf16 2x mode) runs before every 1x tensor_reduce.

Engine split:
  ACT    : exp+relu of q/k (bias fused), v copy (+bias), A^T copy
  GPSIMD : fm adds (min(exp,1)+relu), kbd/abd mask*broadcast products
  DVE    : min(exp,1), halving adds + segmented reduces, fast recip,
           products against expander outputs
  PE     : projections (fp8 DR), ZD/A small matmuls, Z/A expanders
"""

import numpy as np
import ml_dtypes

import concourse.bass as bass
import concourse.bacc as bacc
import concourse.mybir as mybir
import concourse.tile as tile
from concourse.bass_utils import run_bass_kernel_spmd
from concourse.dve_ops import RECIP_APPROX_FAST_CONSTS, RECIPROCAL_APPROX_FAST

F32 = mybir.dt.float32
BF16 = mybir.dt.bfloat16
FP8 = mybir.dt.float8e4
AF = mybir.ActivationFunctionType
ALU = mybir.AluOpType
DR = mybir.MatmulPerfMode.DoubleRow


NCORES = 8
NH, DH, HID = 8, 16, 128
S, NTOK, C = 77, 80, 256
SP = 80                          # padded k-side pitch (aligned halving)
B, N_, H_, W_ = 4, 80, 24, 24
NTOTAL = B * H_ * W_            # 2304
NLOC = NTOTAL // NCORES         # 288
GRP = 6                         # batch elements per group
NGRP_SUPER = 4                  # groups per supergroup (4 x 8 rows in PSUM bands)
SUPER = GRP * NGRP_SUPER        # 24 n per supergroup


def build_nc(n_loc: int = NLOC) -> bass.Bass:
    assert n_loc % SUPER == 0
    nsuper = n_loc // SUPER

    nc = bacc.Bacc()

    qt = nc.declare_dram_parameter("qt", [2, HID, n_loc * NTOK], FP8, isOutput=False)
    xg = nc.declare_dram_parameter("xg", [2, HID, n_loc * S], FP8, isOutput=False)
    xt = nc.declare_dram_parameter("xt", [HID, n_loc * S], BF16, isOutput=False)
    c8p = nc.declare_dram_parameter("c8", [HID, 4 * HID], FP8, isOutput=False)
    CBW = 2 * HID + 2 * SUPER * NH + 16   # wv|e8|m1|mn|f32bits
    cbp = nc.declare_dram_parameter("cb", [HID, CBW], BF16, isOutput=False)
    out = nc.declare_dram_parameter("o", [HID, n_loc], F32, isOutput=True)

    QF = GRP * NTOK   # 480 free elems per group (q side)
    KF = GRP * S      # 462 free elems per group (k/v side)
    QS = SUPER * NTOK  # 1920 per supergroup
    KS = SUPER * S     # 1848

    rc = RECIP_APPROX_FAST_CONSTS

    with tile.TileContext(nc) as tc:
        with (
            tc.tile_pool(name="consts", bufs=1) as consts,
            tc.tile_pool(name="dmain", bufs=3) as dmain,
            tc.tile_pool(name="work", bufs=6) as work,
            tc.tile_pool(name="persist", bufs=3) as persist,
            tc.tile_pool(name="prod", bufs=2) as prod,
            tc.tile_pool(name="za", bufs=3) as za,
            tc.tile_pool(name="small", bufs=8) as small,
            tc.tile_pool(name="outp", bufs=1) as outp,
            tc.tile_pool(name="pq", bufs=1, space="PSUM") as pqp,
            tc.tile_pool(name="pk", bufs=1, space="PSUM") as pkp,
            tc.tile_pool(name="pv", bufs=1, space="PSUM") as pvp,
            tc.tile_pool(name="pzd", bufs=1, space="PSUM") as pzdp,
            tc.tile_pool(name="pe", bufs=3, space="PSUM") as pep,
            tc.tile_pool(name="pa", bufs=1, space="PSUM") as pap,
        ):
            # ---- constants: two packed blobs, two DMAs ----
            cb_t = consts.tile([HID, CBW], BF16)
            nc.sync.dma_start(cb_t[:], cbp[:])
            c8_t = consts.tile([HID, 4 * HID], FP8)
            nc.sync.dma_start(c8_t[:], c8p[:])
            # DoubleRow stationaries: [Ki=128, Ko=2, M=128]
            wq_dr = c8_t[:, 0:2 * HID].rearrange("p (ko m) -> p ko m", m=HID)
            wk_dr = c8_t[:, 2 * HID:4 * HID].rearrange("p (ko m) -> p ko m", m=HID)
            wv_t = cb_t[:, 0:HID]
            e8_t = cb_t[:, HID:2 * HID]
            m1_t = cb_t[:, 2 * HID:2 * HID + SUPER * NH]
            mn_t = cb_t[:, 2 * HID + SUPER * NH:2 * HID + 2 * SUPER * NH]
            fb_t = cb_t[:, CBW - 16:CBW].bitcast(F32)
            bq_t = fb_t[:, 0:1]
            bk_t = fb_t[:, 1:2]
            bv_t = fb_t[:, 2:3]
            bq1_t = fb_t[:, 3:4]   # bq + 1
            bk1_t = fb_t[:, 4:5]   # bk + 1
            tc.strict_bb_all_engine_barrier()

            # HAM warm-up: chained garbage matmuls keep the PE busy >3.4us so
            # the clock gate opens before real work; overlaps the first input
            # DMA. (Steady state re-throttles regardless; this helps the head.)
            heat = pep.tile([HID, 512], F32, tag="pze")
            for h in range(14):
                nc.tensor.matmul(heat[:], c8_t[:, 0:HID], c8_t[:, 0:512],
                                 start=(h == 0), stop=(h == 13))

            outT = outp.tile([HID, n_loc], F32)

            for sg in range(nsuper):
                # ---- supergroup DMA in (fp8) ----
                qt_sb = dmain.tile([HID, 2, QS], FP8, tag="qt")
                kg_sb = dmain.tile([HID, 2, KS], FP8, tag="kg")
                xt_sb = dmain.tile([HID, KS], BF16, tag="xt")
                nc.sync.dma_start(xt_sb[:], xt[:, sg * KS:(sg + 1) * KS])
                for po in range(2):
                    nc.sync.dma_start(
                        qt_sb[:, po, :], qt[po, :, sg * QS:(sg + 1) * QS]
                    )
                    nc.sync.dma_start(
                        kg_sb[:, po, :], xg[po, :, sg * KS:(sg + 1) * KS]
                    )

                # supergroup-lifetime PSUM bands (4 groups x 8 rows each)
                pzd = pzdp.tile([HID, 512], F32, tag="pzd")
                pzd = pzd[:, :QF]
                pa = pap.tile([HID, 512], F32, tag="pa")
                pa = pa[:, :KF]

                # supergroup-wide epilogue tiles; kfm uses an 80-token pitch
                # (cols S..SP-1 zeroed) so halving adds stay 4B-aligned.
                qfm = persist.tile([HID, SUPER, NTOK], BF16, tag="qfm")
                kfm = persist.tile([HID, SUPER, SP], BF16, tag="kfm")
                vsb = persist.tile([HID, KS], BF16, tag="vsb")
                nc.vector.memset(kfm[:, :, S:SP], 0.0)

                # ================= front half: proj + fm =================
                for g in range(NGRP_SUPER):
                    qs = slice(g * QF, (g + 1) * QF)
                    ks = slice(g * KF, (g + 1) * KF)
                    gn = slice(g * GRP, (g + 1) * GRP)

                    pq = pqp.tile([HID, 512], F32, tag="pq")
                    pq = pq[:, :QF]
                    pk = pkp.tile([HID, 512], F32, tag="pk")
                    pk = pk[:, :KF]
                    pv = pvp.tile([HID, 512], F32, tag="pv")
                    pv = pv[:, :KF]
                    nc.tensor.matmul(pq[:], wq_dr, qt_sb[:, :, qs],
                                     start=True, stop=True, perf_mode=DR)
                    nc.tensor.matmul(pk[:], wk_dr, kg_sb[:, :, ks],
                                     start=True, stop=True, perf_mode=DR)
                    nc.tensor.matmul(pv[:], wv_t, xt_sb[:, ks],
                                     start=True, stop=True)

                    # feature map: fm(y) = min(exp(y), 1) + relu(y)
                    eq = work.tile([HID, QF], BF16, tag="eq")
                    rq = work.tile([HID, QF], BF16, tag="rq")
                    ek = work.tile([HID, KF], BF16, tag="ek")
                    rk = work.tile([HID, KF], BF16, tag="rk")
                    nc.scalar.activation(eq[:], pq[:], AF.Exp, bias=bq_t)
                    nc.scalar.activation(rq[:], pq[:], AF.Relu, bias=bq_t)
                    nc.scalar.activation(ek[:], pk[:], AF.Exp, bias=bk_t)
                    nc.scalar.activation(rk[:], pk[:], AF.Relu, bias=bk_t)
                    nc.scalar.activation(
                        vsb[:, ks].rearrange("p (g s) -> p g s", s=S),
                        pv[:].rearrange("p (g s) -> p g s", s=S),
                        AF.Identity, bias=bv_t)
                    eqm = work.tile([HID, QF], BF16, tag="eqm")
                    ekm = work.tile([HID, KF], BF16, tag="ekm")
                    nc.vector.tensor_scalar_min(eqm[:], eq[:], 1.0)
                    nc.vector.tensor_scalar_min(ekm[:], ek[:], 1.0)
                    nc.gpsimd.tensor_tensor(
                        qfm[:, gn, :].rearrange("p g l -> p (g l)"),
                        eqm[:], rq[:], ALU.add)
                    nc.gpsimd.tensor_tensor(
                        kfm[:, gn, 0:S],
                        ekm[:].rearrange("p (g s) -> p g s", s=S),
                        rk[:].rearrange("p (g s) -> p g s", s=S), ALU.add)

                # ===== supergroup: Ksum via halving add + one reduce =====
                kh = prod.tile([HID, SUPER, SP // 2], BF16, tag="kh")
                ksum = small.tile([HID, SUPER], F32, tag="ksum")
                nc.vector.tensor_tensor(
                    kh[:], kfm[:, :, 0:SP // 2], kfm[:, :, SP // 2:SP],
                    ALU.add)
                nc.vector.tensor_reduce(
                    ksum[:], kh[:], mybir.AxisListType.X, ALU.add)
                # KBD = mask1 * Ksum  (8 cols per n, whole supergroup)
                kbd = small.tile([HID, SUPER * NH], BF16, tag="kbd")
                nc.gpsimd.tensor_tensor(
                    kbd[:].rearrange("p (g h) -> p g h", h=NH),
                    m1_t[:].rearrange("p (g h) -> p g h", h=NH),
                    ksum[:, :, None].to_broadcast((HID, SUPER, NH)),
                    ALU.mult)

                # ZD rows -> 8-row bands at partition base 32*g
                for gi in range(SUPER):
                    g = gi // GRP
                    i = gi % GRP
                    nc.tensor.matmul(
                        pzd[32 * g:32 * g + NH, i * NTOK:(i + 1) * NTOK],
                        kbd[:, gi * NH:(gi + 1) * NH],
                        qfm[:, gi, :],
                        start=True, stop=True, skip_group_check=True,
                        tile_position=(0, 32 * g))

                # ========== supergroup: Z ~= 1/ZD (fast recip, bf16 out) =====
                zpk = za.tile([HID, QF], BF16, tag="zpk")
                nc.vector._custom_dve(
                    RECIPROCAL_APPROX_FAST, out=zpk[:], in0=pzd[:],
                    s0=rc["s0"], s1=rc["s1"], imm2=rc["imm2"])

                # ================= back half =================
                prodq = prod.tile([HID, SUPER, NTOK], BF16, tag="prodq")
                for g in range(NGRP_SUPER):
                    gn = slice(g * GRP, (g + 1) * GRP)
                    rowg = slice(32 * g, 32 * g + NH)
                    # Zexp (128, 480): one expander matmul per group
                    pze = pep.tile([HID, 512], F32, tag="pze")
                    pze = pze[:, :QF]
                    nc.tensor.matmul(
                        pze[:], e8_t[rowg, :], zpk[rowg, :],
                        start=True, stop=True, tile_position=(32 * g, 0))
                    nc.vector.tensor_tensor(
                        prodq[:, gn, :],
                        qfm[:, gn, :],
                        pze[:].rearrange("p (g l) -> p g l", l=NTOK),
                        ALU.mult)

                # Qbar: halving add + one reduce for the whole supergroup
                qh = prod.tile([HID, SUPER, NTOK // 2], BF16, tag="qh")
                qbar = small.tile([HID, SUPER], F32, tag="qbar")
                nc.vector.tensor_tensor(
                    qh[:], prodq[:, :, 0:NTOK // 2], prodq[:, :, NTOK // 2:],
                    ALU.add)
                nc.vector.tensor_reduce(
                    qbar[:], qh[:], mybir.AxisListType.X, ALU.add)

                # Abd = maskn * Qbar (whole supergroup)
                abd = small.tile([HID, SUPER * NH], BF16, tag="abd")
                nc.gpsimd.tensor_tensor(
                    abd[:].rearrange("p (g h) -> p g h", h=NH),
                    mn_t[:].rearrange("p (g h) -> p g h", h=NH),
                    qbar[:, :, None].to_broadcast((HID, SUPER, NH)),
                    ALU.mult)

                # A^T rows -> 8-row bands at partition base 32*g
                for gi in range(SUPER):
                    g = gi // GRP
                    i = gi % GRP
                    nc.tensor.matmul(
                        pa[32 * g:32 * g + NH, i * S:(i + 1) * S],
                        abd[:, gi * NH:(gi + 1) * NH],
                        kfm[:, gi, 0:S],
                        start=True, stop=True, skip_group_check=True,
                        tile_position=(0, 32 * g))

                # A^T -> SBUF bf16 once per supergroup
                apk = za.tile([HID, KF], BF16, tag="apk")
                nc.scalar.activation(apk[:], pa[:], AF.Copy)

                prodv = prod.tile([HID, SUPER, SP], BF16, tag="prodv")
                nc.vector.memset(prodv[:, :, S:SP], 0.0)
                for g in range(NGRP_SUPER):
                    gn = slice(g * GRP, (g + 1) * GRP)
                    ks = slice(g * KF, (g + 1) * KF)
                    rowg = slice(32 * g, 32 * g + NH)
                    pae = pep.tile([HID, 512], F32, tag="pze")
                    pae = pae[:, :KF]
                    nc.tensor.matmul(
                        pae[:], e8_t[rowg, :], apk[rowg, :],
                        start=True, stop=True, tile_position=(32 * g, 0))
                    nc.vector.tensor_tensor(
                        prodv[:, gn, 0:S],
                        vsb[:, ks].rearrange("p (g s) -> p g s", s=S),
                        pae[:].rearrange("p (g s) -> p g s", s=S),
                        ALU.mult)

                # out: halving add + one reduce for the whole supergroup
                vh = prod.tile([HID, SUPER, SP // 2], BF16, tag="vh")
                nc.vector.tensor_tensor(
                    vh[:], prodv[:, :, 0:SP // 2], prodv[:, :, SP // 2:SP],
                    ALU.add)
                ocol = sg * SUPER
                nc.vector.tensor_reduce(
                    outT[:, ocol:ocol + SUPER], vh[:],
                    mybir.AxisListType.X, ALU.add)

            nc.sync.dma_start(out[:], outT[:])

    nc.finalize()
    return nc


# ---------------- host-side packing ----------------

def make_consts():
    hd = np.arange(HID)
    e8 = (hd[None, :] // DH == (np.arange(HID) % NH)[:, None]).astype(np.float32)
    m1 = np.zeros((HID, SUPER * NH), np.float32)
    for i in range(SUPER):
        for h in range(NH):
            m1[h * DH:(h + 1) * DH, i * NH + h] = 1.0
    mn = (m1 / float(NTOK)).astype(np.float32)
    return e8, m1, mn


def shard_inputs(query, x, guidance, Wq, bq, Wk, bk, Wv, bv, n_loc=NLOC,
                 ncores=NCORES):
    qin = np.ascontiguousarray(
        query.transpose(0, 2, 3, 1, 4)).reshape(NTOTAL, NTOK, C)
    e8, m1, mn = make_consts()
    bf = ml_dtypes.bfloat16
    f8 = ml_dtypes.float8_e4m3
    wqr = Wq.reshape(2, HID, HID)
    wkr = Wk.reshape(2, HID, HID)
    # DoubleRow stationaries [Ki, Ko, M] flattened to [Ki, Ko*M]
    wq_dr = np.stack([wqr[0], wqr[1]], axis=1).reshape(HID, 2 * HID)
    wk_dr = np.stack([wkr[0], wkr[1]], axis=1).reshape(HID, 2 * HID)
    c8 = np.concatenate([wq_dr, wk_dr], axis=1).astype(f8)
    cb = np.concatenate([Wv, e8, m1, mn], axis=1).astype(bf)
    z = np.zeros(HID, np.float32)
    fb = np.stack(
        [bq, bk, bv, bq + 1.0, bk + 1.0, z, z, z], axis=1).astype(np.float32)
    fb_as_bf = np.ascontiguousarray(fb).view(bf)
    cb = np.concatenate([cb, fb_as_bf], axis=1)
    shared = dict(cb=cb, c8=c8)
    in_maps = []
    for i in range(ncores):
        sl = slice(i * n_loc, (i + 1) * n_loc)
        qc = qin[sl].reshape(n_loc * NTOK, C)
        xc = x[sl].reshape(n_loc * S, HID)
        gc = guidance[sl].reshape(n_loc * S, HID)
        m = dict(shared)
        m["qt"] = np.ascontiguousarray(qc.T).reshape(2, HID, n_loc * NTOK).astype(f8)
        xct = np.ascontiguousarray(xc.T)
        xgs = np.stack([xct, np.ascontiguousarray(gc.T)])
        m["xg"] = xgs.astype(f8)
        m["xt"] = xct.astype(bf)
        in_maps.append(m)
    return in_maps


_NC_CACHE = {}


def kernel(**inputs) -> np.ndarray:
    inputs = {k: np.asarray(v, dtype=np.float32) if np.asarray(v).dtype != np.int32
              else np.asarray(v) for k, v in inputs.items()}
    in_maps = shard_inputs(**inputs)
    if NLOC not in _NC_CACHE:
        _NC_CACHE[NLOC] = build_nc(NLOC)
    nc = _NC_CACHE[NLOC]
    res = run_bass_kernel_spmd(nc, in_maps, core_ids=list(range(NCORES)))
    outs = [np.asarray(res.results[i]["o"]).T for i in range(NCORES)]
    full = np.concatenate(outs, axis=0)  # (2304, 128)
    return full.reshape(B, H_, W_, HID).astype(np.float32)
